# revision 1
# baseline (speedup 1.0000x reference)
"""Trainium2 Bass kernel for MultiHeadSelfAttention (cross-attention variant).

Problem: B=2, LQ=LK=2048, D=1024, H=16, d_k=64, fp32.
  q_a = cdd @ W_q + b_q ; k_a = his @ W_k + b_k ; v_a = his @ W_v + b_v
  S = q k^T / 8 ; A = exp(S) / (sum_k exp(S) + 1e-8) ; ctx = A v
  returns (context, q_a)

Sharding (8 cores, no collectives): core c handles batch c//4 and head-block
c%4 (4 heads = 256 columns of W_q/W_k/W_v).  Each core writes disjoint column
slices of both outputs; the host gathers them.

Design (ACT-exp is the roofline: 16.8M exps/core @ 1 elem/lane/cyc
@1.2GHz ~= 110us; fp32 PSUM caps exp FD at 1024 -> ~128us on ACT alone):
  - host pre-transposes cdd/his to feature-major [D, L] bf16; X^T tiles and
    bf16 weights DMA straight into SBUF -> ZERO PE transposes, ZERO scalar
    copies; the ACT engine runs exp and nothing else.
  - Q^T/K^T projections: W stationary, X^T moving (bf16 full-rate), PSUM
    accumulate over 8 feature tiles, DVE bias-add into persistent QT/KT
    (f32r, which the verifier requires be produced by rounding instructions).
  - V natural [tokens, 65] bf16 (ones column gives attention row-sums free).
  - scores: per ct, heads 2ct/2ct+1 live at partitions 0-63/64-127; the two
    [64,128]x[64,512] score matmuls auto-derive tile_position (0,0)/(64,0)
    and run CONCURRENTLY in the PE array (row tiling) -> 2x score rate.
  - exp: one FD=1024 instruction per (ct,kt) group over both heads' scores;
    every 4th group runs on the otherwise-idle DVE as a Schraudolph
    bit-trick exp (bf16 bits = round(A*s + B), ~+-5% sawtooth on 1/4 of the
    keys -> ctx max-err ~1.4e-2 < 2e-2), cutting ACT from ~128us to ~96us.
  - MM2: [V|1]^T @ expS^T (bf16) accumulated per head into cpA/cpB PSUM
    banks, emitted with a lag (software pipeline) so PE never waits on ACT.
  - normalize: DVE copy -> PE transpose [65,128] blocks -> per-partition
    reciprocal -> scale into natural-layout ctx tiles -> DMA.
  - q_a is DMA'd out transposed [COLS, L] and un-transposed on host.
  - PSUM: sp 2x2 banks + cpA + cpB + pp + tp = 8 exactly.
  - emission: KV chunk 0 + Q chunk 0 eager; KV 1-3 and Q one-ahead flow as
    fine-grained units into attention's PE idle slots, with drain_until()
    forcing emission-order prerequisites (Tile deps are emission-ordered).
Measured ~118-125us/core (slope method) vs 158us baseline.
"""

import numpy as np
from contextlib import ExitStack

B = 2
L = 2048
D = 1024
H = 16
DK = 64
P = 128
NCORES = 8
CPB = 4  # cores per batch
HPC = H // CPB  # heads per core = 4
COLS = HPC * DK  # 256 output columns per core
CHUNK = 512  # token chunk (max fp32 moving operand)

_CACHE = {}


def _build(cfg=None, repeat=1):
    import concourse.tile as tile
    from concourse import bacc, masks, mybir

    f32 = mybir.dt.float32
    f32r = mybir.dt.float32r
    bf16 = mybir.dt.bfloat16
    i16 = mybir.dt.int16
    Exp = mybir.ActivationFunctionType.Exp
    add_op = mybir.AluOpType.add
    mult_op = mybir.AluOpType.mult

    FT = D // P  # feature tiles = 8
    TT = L // P  # key tiles = 16
    TCH = L // CHUNK  # token chunks = 4
    CT = COLS // P  # column tiles = 2
    IT = CHUNK // P  # token tiles per chunk = 4
    VW = DK + 1  # 65: V columns + ones column

    cfg = dict(
        dict(lag=6, es=8, feedn=3, kvbufs=3, dve_every=4, dve_off=3, pair=True),
        **(cfg or {}),
    )
    LAG = cfg["lag"]
    NE = cfg["dve_every"]  # every NE-th exp group runs on DVE (Schraudolph)
    # Schraudolph exp in bf16 bit space: bits = round(A*s + B) makes
    # bitcast_bf16(bits) ~= exp(s/8) with ~+-3% sawtooth error.
    # A = 2^7 * log2(e) / 8; B = 127*2^7 - C, C = 5.57 centers the error.
    SCH_A = 16.0 * 1.4426950408889634
    SCH_B = 16256.0 - 5.57

    nc = bacc.Bacc(
        "TRN2",
        target_bir_lowering=False,
        debug=False,
        num_devices=NCORES,
    )

    x_qt = nc.dram_tensor("x_qt", [D, L], bf16, kind="ExternalInput").ap()
    x_kvt = nc.dram_tensor("x_kvt", [D, L], bf16, kind="ExternalInput").ap()
    w_q = nc.dram_tensor("w_q", [D, COLS], bf16, kind="ExternalInput").ap()
    w_k = nc.dram_tensor("w_k", [D, COLS], bf16, kind="ExternalInput").ap()
    w_v = nc.dram_tensor("w_v", [D, COLS], bf16, kind="ExternalInput").ap()
    b_q = nc.dram_tensor("b_q", [COLS], f32, kind="ExternalInput").ap()
    b_k = nc.dram_tensor("b_k", [COLS], f32, kind="ExternalInput").ap()
    b_v = nc.dram_tensor("b_v", [COLS], f32, kind="ExternalInput").ap()
    q_outt = nc.dram_tensor("q_outt", [COLS, L], f32, kind="ExternalOutput").ap()
    c_out = nc.dram_tensor("c_out", [L, COLS], f32, kind="ExternalOutput").ap()

    with tile.TileContext(nc) as tc, ExitStack() as ctx:
        singles = ctx.enter_context(tc.tile_pool(name="singles", bufs=1))

        identity = singles.tile([P, P], f32)
        masks.make_identity(nc, identity[:])

        # biases: q/k as per-partition scalars in ^T layout; v broadcast to rows
        bq_sb = singles.tile([P, CT], f32)
        bk_sb = singles.tile([P, CT], f32)
        nc.sync.dma_start(bq_sb[:], b_q.rearrange("(c p) -> p c", p=P))
        nc.sync.dma_start(bk_sb[:], b_k.rearrange("(c p) -> p c", p=P))
        bv_row = singles.tile([1, COLS], f32)
        nc.sync.dma_start(bv_row[:], b_v.rearrange("(o c) -> o c", o=1))
        bv_bcast = singles.tile([P, COLS], f32)
        nc.gpsimd.partition_broadcast(bv_bcast[:], bv_row[:1])

        # weights [D, COLS] -> [128, FT, COLS], bf16 straight from host (the
        # BIR verifier requires f32r operands to come from a rounding
        # instruction, so fp32-rate mode would cost DVE round-trips; bf16
        # matmuls run at the same PE rate and halve DMA+SBUF).
        wq_sb = singles.tile([P, FT * COLS], bf16, name="wq_sb").rearrange(
            "p (f c) -> p f c", f=FT
        )
        wk_sb = singles.tile([P, FT * COLS], bf16, name="wk_sb").rearrange(
            "p (f c) -> p f c", f=FT
        )
        wv_sb = singles.tile([P, FT * COLS], bf16, name="wv_sb").rearrange(
            "p (f c) -> p f c", f=FT
        )
        for wsb, wdr in ((wq_sb, w_q), (wk_sb, w_k), (wv_sb, w_v)):
            nc.sync.dma_start(wsb[:], wdr.rearrange("(f p) c -> p f c", p=P))

        # persistent attention operands
        QT = singles.tile([P, CT * L], f32r, name="QT").rearrange(
            "p (c l) -> p c l", c=CT
        )
        KT = singles.tile([P, CT * L], f32r, name="KT").rearrange(
            "p (c l) -> p c l", c=CT
        )
        V = singles.tile([P, TT * HPC * VW], bf16, name="V").rearrange(
            "p (t h w) -> p t h w", t=TT, h=HPC
        )
        ones1 = singles.tile([P, 1], f32)
        nc.vector.memset(ones1[:], 1.0)
        nc.vector.tensor_copy(
            V[:, :, :, DK : DK + 1], ones1[:].to_broadcast((P, TT, HPC, 1))
        )

        # ---- pools ----
        # SBUF
        xt_pool = ctx.enter_context(tc.tile_pool(name="xt", bufs=2))
        espool = ctx.enter_context(tc.tile_pool(name="es", bufs=cfg["es"]))
        ctpool = ctx.enter_context(tc.tile_pool(name="ct", bufs=2))
        ctxpool = ctx.enter_context(tc.tile_pool(name="ctxsb", bufs=1))
        recpool = ctx.enter_context(tc.tile_pool(name="rec", bufs=4))
        # PSUM: sp 2x2 + cpA 1 + cpB 1 + pp 1 + tp 1 = 8 banks
        spool = ctx.enter_context(tc.tile_pool(name="spool", bufs=2, space="PSUM"))
        cpoolA = ctx.enter_context(tc.tile_pool(name="cpoolA", bufs=1, space="PSUM"))
        cpoolB = ctx.enter_context(tc.tile_pool(name="cpoolB", bufs=1, space="PSUM"))
        ppsum = ctx.enter_context(tc.tile_pool(name="ppsum", bufs=1, space="PSUM"))
        tpsum = ctx.enter_context(tc.tile_pool(name="tpsum", bufs=1, space="PSUM"))

        def load_x(xdram, tag, ch, bufs):
            """DMA a 512-token chunk of host-pretransposed bf16 X^T into SBUF."""
            tok0 = ch * CHUNK
            xt = xt_pool.tile(
                [P, FT * CHUNK], bf16, tag=f"xt{tag}", name=f"xt{tag}", bufs=bufs
            )
            xt = xt.rearrange("p (f l) -> p f l", f=FT)
            for ft in range(FT):
                nc.sync.dma_start(
                    xt[:, ft, :], xdram[ft * P : (ft + 1) * P, tok0 : tok0 + CHUNK]
                )
            return xt

        def proj_T_ct(wsb, xt, bsb, OUT, ch, ct, units=None):
            """^T-layout projection (columns on partitions) with bias."""
            tok0 = ch * CHUNK
            cell = {}

            def mms(fts):
                if "pp" not in cell:
                    cell["pp"] = ppsum.tile([P, CHUNK], f32, tag="pp", name="pp")
                for ft in fts:
                    nc.tensor.matmul(
                        cell["pp"][:],
                        wsb[:, ft, ct * P : (ct + 1) * P],
                        xt[:, ft, :],
                        start=(ft == 0),
                        stop=(ft == FT - 1),
                    )

            def fin():
                nc.vector.tensor_scalar_add(
                    OUT[:, ct, tok0 : tok0 + CHUNK], cell["pp"], bsb[:, ct : ct + 1]
                )

            if units is None:
                mms(range(FT))
                fin()
            else:
                for f0 in range(0, FT, 2):
                    units.append(lambda f=f0: mms((f, f + 1)))
                units.append(fin)

        def proj_v_it(xt, ch, it, units=None):
            """natural-layout V projection (tokens on partitions) with bias."""
            cell = {}

            def mms(fts):
                if "pv" not in cell:
                    cell["pv"] = ppsum.tile([P, COLS], f32, tag="pp", name="pv")
                for ft in fts:
                    nc.tensor.matmul(
                        cell["pv"][:],
                        xt[:, ft, it * P : (it + 1) * P],
                        wv_sb[:, ft, :],
                        start=(ft == 0),
                        stop=(ft == FT - 1),
                    )

            def fin():
                nc.vector.tensor_tensor(
                    V[:, ch * IT + it, :, 0:DK],
                    cell["pv"][:].rearrange("p (h w) -> p h w", h=HPC),
                    bv_bcast[:].rearrange("p (h w) -> p h w", h=HPC),
                    op=add_op,
                )

            if units is None:
                mms(range(FT))
                fin()
            else:
                units.append(lambda: mms(range(0, 4)))
                units.append(lambda: mms(range(4, FT)))
                units.append(fin)

        def qout_dma(ch, ct):
            tok0 = ch * CHUNK
            nc.sync.dma_start(
                q_outt[ct * P : (ct + 1) * P, tok0 : tok0 + CHUNK],
                QT[:, ct, tok0 : tok0 + CHUNK].bitcast(f32),
            )

        def kv_chunk(ch, units=None):
            xt = load_x(x_kvt, "kv", ch, cfg["kvbufs"])
            for ct in range(CT):
                proj_T_ct(wk_sb, xt, bk_sb, KT, ch, ct, units)
            for it in range(IT):
                proj_v_it(xt, ch, it, units)

        def q_chunk(ch, units=None):
            xt = load_x(x_qt, "q", ch, 2)
            for ct in range(CT):
                proj_T_ct(wq_sb, xt, bq_sb, QT, ch, ct, units)
                if units is None:
                    qout_dma(ch, ct)
                else:
                    units.append(lambda c=ct: qout_dma(ch, c))

        def normalize(cp, h, ctx_tiles):
            """cp[:VW] = [ctx^T | sums] for head h -> scaled natural ctx."""
            ctT = ctpool.tile([P, CHUNK], f32, tag="ctT", name="ctT")
            nc.vector.tensor_copy(ctT[:VW, :], cp[:VW, :])
            for it in range(IT):
                t2 = tpsum.tile([P, CHUNK], f32, tag="tp", name="t2")
                nc.tensor.transpose(
                    t2[:, :VW],
                    ctT[:VW, it * P : (it + 1) * P],
                    identity[:VW, :VW],
                )
                rec = recpool.tile([P, 2], f32, tag="rec", name="rec")
                nc.vector.tensor_scalar_add(rec[:, 0:1], t2[:, DK : DK + 1], 1e-8)
                nc.vector.reciprocal(rec[:, 1:2], rec[:, 0:1])
                nc.vector.tensor_scalar_mul(
                    ctx_tiles[it][:, h * DK : (h + 1) * DK],
                    t2[:, 0:DK],
                    rec[:, 1:2],
                )

        def attention(qc, feed, drain_until, k_fin, v_fin, q_fin):
            q0 = qc * CHUNK
            ctx_tiles = [
                ctxpool.tile([P, COLS], f32, tag=f"ctx{it}", name=f"ctxt{it}")
                for it in range(IT)
            ]
            for ct in range(CT):
                hA, hB = 2 * ct, 2 * ct + 1
                drain_until(q_fin[qc][ct])
                cpA = cpoolA.tile([P, CHUNK], f32, tag="cpA", name="cpA")
                cpB = cpoolB.tile([P, CHUNK], f32, tag="cpB", name="cpB")

                def mm2(kt, es):
                    # V tile kt must be projected (emitted) before this read
                    drain_until(v_fin[kt])
                    nc.tensor.matmul(
                        cpA[:VW, :],
                        V[:, kt, hA, :],
                        es[:, 0:CHUNK],
                        start=(kt == 0),
                        stop=(kt == TT - 1),
                    )
                    nc.tensor.matmul(
                        cpB[:VW, :],
                        V[:, kt, hB, :],
                        es[:, CHUNK : 2 * CHUNK],
                        start=(kt == 0),
                        stop=(kt == TT - 1),
                    )

                def expify(es, sp, gi):
                    # dve_off=1 keeps the DVE-exp groups away from the
                    # end-of-pass normalize burst (also DVE) at kt=15
                    if NE and gi % NE == cfg["dve_off"]:
                        # DVE Schraudolph exp: bf16 bits via fused mul-add
                        nc.vector.tensor_scalar(
                            es[:].bitcast(i16),
                            sp[:],
                            SCH_A,
                            SCH_B,
                            op0=mult_op,
                            op1=add_op,
                        )
                    else:
                        nc.scalar.activation(es[:], sp[:], Exp, scale=0.125)

                if cfg["pair"]:
                    pend = []
                    for kt in range(TT):
                        # K^T for key-chunk kt//4, tile ct, must be emitted
                        drain_until(k_fin[kt // 4][ct])
                        sp = spool.tile([P, 2 * CHUNK], f32, tag="sp", name="sp")
                        # two concurrent row-tiled score matmuls (tile_position
                        # auto-derives (0,0) and (64,0) from base partitions)
                        nc.tensor.matmul(
                            sp[:, 0:CHUNK],
                            KT[0:DK, ct, kt * P : (kt + 1) * P],
                            QT[0:DK, ct, q0 : q0 + CHUNK],
                            start=True,
                            stop=True,
                        )
                        nc.tensor.matmul(
                            sp[:, CHUNK : 2 * CHUNK],
                            KT[DK:P, ct, kt * P : (kt + 1) * P],
                            QT[DK:P, ct, q0 : q0 + CHUNK],
                            start=True,
                            stop=True,
                        )
                        es = espool.tile([P, 2 * CHUNK], bf16, tag="es", name="es")
                        expify(es, sp, kt)
                        pend.append((kt, es))
                        if len(pend) > LAG:
                            mm2(*pend.pop(0))
                        feed()
                    while pend:
                        mm2(*pend.pop(0))
                else:
                    # A/B variant: per-head serial scores (no row-tile pairing)
                    for h, cp, rows in (
                        (hA, cpA, slice(0, DK)),
                        (hB, cpB, slice(DK, P)),
                    ):
                        def mm2s(g, es):
                            for j in range(2):
                                kt = 2 * g + j
                                drain_until(v_fin[kt])
                                nc.tensor.matmul(
                                    cp[:VW, :],
                                    V[:, kt, h, :],
                                    es[:, j * CHUNK : (j + 1) * CHUNK],
                                    start=(kt == 0),
                                    stop=(kt == TT - 1),
                                )

                        pend = []
                        for g in range(TT // 2):
                            drain_until(k_fin[(2 * g + 1) // 4][ct])
                            sp = spool.tile([P, 2 * CHUNK], f32, tag="sp", name="sp")
                            for j in range(2):
                                kt = 2 * g + j
                                nc.tensor.matmul(
                                    sp[:, j * CHUNK : (j + 1) * CHUNK],
                                    KT[rows, ct, kt * P : (kt + 1) * P],
                                    QT[rows, ct, q0 : q0 + CHUNK],
                                    start=True,
                                    stop=True,
                                )
                            es = espool.tile([P, 2 * CHUNK], bf16, tag="es", name="es")
                            expify(es, sp, g)
                            pend.append((g, es))
                            if len(pend) > LAG:
                                mm2s(*pend.pop(0))
                            feed()
                        while pend:
                            mm2s(*pend.pop(0))
                normalize(cpA, hA, ctx_tiles)
                normalize(cpB, hB, ctx_tiles)
            for it in range(IT):
                nc.sync.dma_start(
                    c_out[q0 + it * P : q0 + (it + 1) * P, :], ctx_tiles[it][:]
                )

        def emit_all():
            units = []
            state = {"popped": 0}

            def pop1():
                units.pop(0)()
                state["popped"] += 1

            def feed():
                for _ in range(cfg["feedn"]):
                    if units:
                        pop1()

            def drain_until(idx):
                """Ensure the unit with absolute index idx has been emitted."""
                while idx is not None and state["popped"] <= idx:
                    pop1()

            def mark():
                return len(units) + state["popped"] - 1  # index of last appended

            # fill: K0 + Q0 eagerly (they gate the first scores/exp), V0 after
            xt0 = load_x(x_kvt, "kv", 0, cfg["kvbufs"])
            for ct in range(CT):
                proj_T_ct(wk_sb, xt0, bk_sb, KT, 0, ct)
            q_chunk(0)
            for it in range(IT):
                proj_v_it(xt0, 0, it)
            # producer bookkeeping: fin indices (None = already emitted eagerly)
            k_fin = [[None, None] for _ in range(TCH)]
            v_fin = [None] * TT
            q_fin = [[None, None] for _ in range(TCH)]
            # feed queue: remaining KV chunks (all needed within attention
            # chunk 0), then Q chunks one-ahead.
            for ch in range(1, TCH):
                xt = load_x(x_kvt, "kv", ch, cfg["kvbufs"])
                for ct in range(CT):
                    proj_T_ct(wk_sb, xt, bk_sb, KT, ch, ct, units)
                    k_fin[ch][ct] = mark()
                for it in range(IT):
                    proj_v_it(xt, ch, it, units)
                    v_fin[ch * IT + it] = mark()
            for qc in range(TCH):
                nq = qc + 1
                if nq < TCH:
                    xt = load_x(x_qt, "q", nq, 2)
                    for ct in range(CT):
                        proj_T_ct(wq_sb, xt, bq_sb, QT, nq, ct, units)
                        q_fin[nq][ct] = mark()
                        units.append(lambda c=ct, n=nq: qout_dma(n, c))
                attention(qc, feed, drain_until, k_fin, v_fin, q_fin)
            while units:
                pop1()

        for _rep in range(repeat):
            emit_all()
    nc.compile()
    return nc


def _get_nc():
    if "nc" not in _CACHE:
        _CACHE["nc"] = _build()
    return _CACHE["nc"]


def make_in_maps(cdd, his, W_q, b_q, W_k, b_k, W_v, b_v):
    cdd = np.asarray(cdd, dtype=np.float32)
    his = np.asarray(his, dtype=np.float32)
    W_q = np.asarray(W_q, dtype=np.float32)
    W_k = np.asarray(W_k, dtype=np.float32)
    W_v = np.asarray(W_v, dtype=np.float32)
    b_q = np.asarray(b_q, dtype=np.float32)
    b_k = np.asarray(b_k, dtype=np.float32)
    b_v = np.asarray(b_v, dtype=np.float32)
    import ml_dtypes

    bf16 = ml_dtypes.bfloat16
    # feature-major bf16 inputs, shared across the 4 cores of each batch
    cddT = [np.ascontiguousarray(cdd[b].T).astype(bf16) for b in range(B)]
    hisT = [np.ascontiguousarray(his[b].T).astype(bf16) for b in range(B)]
    W_q = W_q.astype(bf16)
    W_k = W_k.astype(bf16)
    W_v = W_v.astype(bf16)
    in_maps = []
    for c in range(NCORES):
        b, hb = divmod(c, CPB)
        sl = slice(hb * COLS, (hb + 1) * COLS)
        in_maps.append(
            {
                "x_qt": cddT[b],
                "x_kvt": hisT[b],
                "w_q": np.ascontiguousarray(W_q[:, sl]),
                "w_k": np.ascontiguousarray(W_k[:, sl]),
                "w_v": np.ascontiguousarray(W_v[:, sl]),
                "b_q": np.ascontiguousarray(b_q[sl]),
                "b_k": np.ascontiguousarray(b_k[sl]),
                "b_v": np.ascontiguousarray(b_v[sl]),
            }
        )
    return in_maps


def assemble_outputs(results):
    context = np.zeros((B, L, D), dtype=np.float32)
    q_a = np.zeros((B, L, D), dtype=np.float32)
    for c, out in enumerate(results):
        b, hb = divmod(c, CPB)
        sl = slice(hb * COLS, (hb + 1) * COLS)
        q_a[b, :, sl] = out["q_outt"].T
        context[b, :, sl] = out["c_out"]
    return (context, q_a)


def kernel(cdd, his, W_q, b_q, W_k, b_k, W_v, b_v):
    from concourse.bass_utils import run_bass_kernel_spmd

    nc = _get_nc()
    in_maps = make_in_maps(cdd, his, W_q, b_q, W_k, b_k, W_v, b_v)

    res = run_bass_kernel_spmd(
        nc, in_maps, core_ids=list(range(NCORES)), trace=_CACHE.get("trace", False)
    )
    _CACHE["last_result"] = res
    return assemble_outputs(res.results)



# revision 45
# speedup vs baseline: 1.9211x; 1.9211x over previous
"""Trainium2 Bass kernel for MultiHeadSelfAttention (cross-attention variant).

Problem: B=2, LQ=LK=2048, D=1024, H=16, d_k=64, fp32.
  q_a = cdd @ W_q + b_q ; k_a = his @ W_k + b_k ; v_a = his @ W_v + b_v
  S = q k^T / 8 ; A = exp(S) / (sum_k exp(S) + 1e-8) ; ctx = A v
  returns (context, q_a)

Sharding (8 cores, no collectives): core c handles batch c//4 and head-block
c%4 (4 heads = 256 columns of W_q/W_k/W_v).  Each core writes disjoint column
slices of both outputs; the host gathers them.

Design (real-HW floor: PE ~123us busy/core = proj 41 + paired scores 27 +
MM2 55; exp = 128 groups of [128,1024] split ACT/DVE):
  - host pre-transposes cdd/his to feature-major [D, L] bf16; X^T tiles and
    bf16 weights DMA straight into SBUF -> ZERO PE transposes, ZERO scalar
    copies; the ACT engine runs exp and nothing else.
  - Q^T/K^T projections: W stationary, X^T moving (bf16 full-rate), PSUM
    accumulate over 8 feature tiles, DVE bias-add into persistent QT/KT
    (f32r, which the verifier requires be produced by rounding instructions).
  - V natural [tokens, 65] bf16 (ones column gives attention row-sums free).
  - scores: per ct, heads 2ct/2ct+1 live at partitions 0-63/64-127; the two
    [64,128]x[64,512] score matmuls auto-derive tile_position (0,0)/(64,0)
    and run CONCURRENTLY in the PE array (row tiling) -> 2x score rate.
  - exp: one FD=1024 instruction per (ct,kt) group over both heads' scores;
    every 4th group runs on the DVE as a Schraudolph bit-trick exp (bf16
    bits = round(A*s + B), ~+-5% sawtooth on 1/4 of the keys -> ctx
    max-err 1.42e-2 < 2e-2).  (A bigger late-phase DVE share measured
    +10us/rep on HW -- the real DVE op is slower than the cost model.)
  - MM2: [V|1]^T @ expS^T (bf16) accumulated per head into cpA/cpB PSUM
    banks; the MM2 backlog (lag) is carried ACROSS ct/qc boundaries so the
    exp pipeline never drains (no refill stalls).
  - normalize: DVE copy -> PE transpose [65,128] blocks -> per-partition
    reciprocal -> scale into natural-layout ctx tiles -> DMA.  (A ^T-layout
    normalize via gpsimd partition_broadcast measured +32us/rep on HW --
    Q7 launch overhead -- and a DMA-engine broadcast serialized badly;
    the PE transposes are the cheapest broadcast in town.)
  - DMA: per-ft x-chunk transfers (real DMA rings parallelize across
    instructions; merging into one instruction per chunk measured +2-4
    us/rep on HW); wk + x_kv chunk 0 are emitted first so the first
    projection starts as early as possible.
  - PSUM: sp 2x2 banks + cpA + cpB + pp + tp = 8 exactly.
  - emission: KV chunk 0 + Q chunk 0 eager; KV 1-3 and Q one-ahead flow as
    fine-grained units into attention's PE idle slots, with drain_until()
    forcing emission-order prerequisites (Tile deps are emission-ordered).
"""

import numpy as np
from contextlib import ExitStack

B = 2
L = 2048
D = 1024
H = 16
DK = 64
P = 128
NCORES = 8
CPB = 4  # cores per batch
HPC = H // CPB  # heads per core = 4
COLS = HPC * DK  # 256 output columns per core
CHUNK = 512  # token chunk (max fp32 moving operand)

_CACHE = {}


def _build(cfg=None, repeat=1):
    import concourse.tile as tile
    from concourse import bacc, masks, mybir

    f32 = mybir.dt.float32
    f32r = mybir.dt.float32r
    bf16 = mybir.dt.bfloat16
    i16 = mybir.dt.int16
    Exp = mybir.ActivationFunctionType.Exp
    add_op = mybir.AluOpType.add
    mult_op = mybir.AluOpType.mult

    FT = D // P  # feature tiles = 8
    TT = L // P  # key tiles = 16
    TCH = L // CHUNK  # token chunks = 4
    CT = COLS // P  # column tiles = 2
    IT = CHUNK // P  # token tiles per chunk = 4
    VW = DK + 1  # 65: V columns + ones column

    cfg = dict(
        # A/B-tested on HW vs the 133922ns baseline (R=8 repeat NEFFs,
        # interleaved batches): a late-qc DVE-exp rebalance (late_from=2,
        # 6/16 on DVE) measured +10us/rep and gpsimd-broadcast normalize
        # +32us/rep -- both reverted.  Merged x/q_outt DMAs (+2-4us/rep;
        # real DMA rings parallelize across instructions) also reverted.
        dict(
            lag=6, es=8, feedn=3, kvbufs=3, dve_every=4, dve_off=3,
            late_from=99, dve_late=(1, 3, 6), xparts=8, wparts=1,
            qoparts=2, pair=True,
        ),
        **(cfg or {}),
    )
    LAG = cfg["lag"]
    NE = cfg["dve_every"]  # every NE-th exp group runs on DVE (Schraudolph)
    # Schraudolph exp in bf16 bit space: bits = round(A*s + B) makes
    # bitcast_bf16(bits) ~= exp(s/8) with ~+-3% sawtooth error.
    # A = 2^7 * log2(e) / 8; B = 127*2^7 - C, C = 5.57 centers the error.
    SCH_A = 16.0 * 1.4426950408889634
    SCH_B = 16256.0 - 5.57

    nc = bacc.Bacc(
        "TRN2",
        target_bir_lowering=False,
        debug=False,
        num_devices=NCORES,
    )

    x_qt = nc.dram_tensor("x_qt", [D, L], bf16, kind="ExternalInput").ap()
    x_kvt = nc.dram_tensor("x_kvt", [D, L], bf16, kind="ExternalInput").ap()
    w_q = nc.dram_tensor("w_q", [D, COLS], bf16, kind="ExternalInput").ap()
    w_k = nc.dram_tensor("w_k", [D, COLS], bf16, kind="ExternalInput").ap()
    w_v = nc.dram_tensor("w_v", [D, COLS], bf16, kind="ExternalInput").ap()
    b_q = nc.dram_tensor("b_q", [COLS], f32, kind="ExternalInput").ap()
    b_k = nc.dram_tensor("b_k", [COLS], f32, kind="ExternalInput").ap()
    b_v = nc.dram_tensor("b_v", [COLS], f32, kind="ExternalInput").ap()
    q_outt = nc.dram_tensor("q_outt", [COLS, L], f32, kind="ExternalOutput").ap()
    c_out = nc.dram_tensor("c_out", [L, COLS], f32, kind="ExternalOutput").ap()

    with tile.TileContext(nc) as tc, ExitStack() as ctx:
        singles = ctx.enter_context(tc.tile_pool(name="singles", bufs=1))

        identity = singles.tile([P, P], f32)
        masks.make_identity(nc, identity[:])

        # weights [D, COLS] -> [128, FT, COLS], bf16 straight from host (the
        # BIR verifier requires f32r operands to come from a rounding
        # instruction, so fp32-rate mode would cost DVE round-trips; bf16
        # matmuls run at the same PE rate and halve DMA+SBUF).  One DMA
        # instruction per tensor (HWDGE costs ~625ns per DMA instruction);
        # wk first -- the eager fill starts with the K projection.
        wq_sb = singles.tile([P, FT * COLS], bf16, name="wq_sb").rearrange(
            "p (f c) -> p f c", f=FT
        )
        wk_sb = singles.tile([P, FT * COLS], bf16, name="wk_sb").rearrange(
            "p (f c) -> p f c", f=FT
        )
        wv_sb = singles.tile([P, FT * COLS], bf16, name="wv_sb").rearrange(
            "p (f c) -> p f c", f=FT
        )
        def load_w(wsb, wdr):
            wdr = wdr.rearrange("(f p) c -> p f c", p=P)
            step = FT // cfg["wparts"]
            for i in range(0, FT, step):
                nc.sync.dma_start(wsb[:, i : i + step, :], wdr[:, i : i + step, :])

        # biases: q/k as per-partition scalars in ^T layout; v broadcast to rows
        bq_sb = singles.tile([P, CT], f32)
        bk_sb = singles.tile([P, CT], f32)
        bv_row = singles.tile([1, COLS], f32)
        bv_bcast = singles.tile([P, COLS], f32)

        def load_biases():
            nc.sync.dma_start(bk_sb[:], b_k.rearrange("(c p) -> p c", p=P))
            nc.sync.dma_start(bq_sb[:], b_q.rearrange("(c p) -> p c", p=P))
            nc.sync.dma_start(bv_row[:], b_v.rearrange("(o c) -> o c", o=1))
            nc.gpsimd.partition_broadcast(bv_bcast[:], bv_row[:1])

        # persistent attention operands
        QT = singles.tile([P, CT * L], f32r, name="QT").rearrange(
            "p (c l) -> p c l", c=CT
        )
        KT = singles.tile([P, CT * L], f32r, name="KT").rearrange(
            "p (c l) -> p c l", c=CT
        )
        V = singles.tile([P, TT * HPC * VW], bf16, name="V").rearrange(
            "p (t h w) -> p t h w", t=TT, h=HPC
        )
        ones1 = singles.tile([P, 1], f32)
        nc.vector.memset(ones1[:], 1.0)
        nc.vector.tensor_copy(
            V[:, :, :, DK : DK + 1], ones1[:].to_broadcast((P, TT, HPC, 1))
        )

        # ---- pools ----
        # SBUF
        xt_pool = ctx.enter_context(tc.tile_pool(name="xt", bufs=2))
        espool = ctx.enter_context(tc.tile_pool(name="es", bufs=cfg["es"]))
        ctpool = ctx.enter_context(tc.tile_pool(name="ct", bufs=2))
        ctxpool = ctx.enter_context(tc.tile_pool(name="ctxsb", bufs=2))
        recpool = ctx.enter_context(tc.tile_pool(name="rec", bufs=4))
        # PSUM: sp 2x2 + cpA 1 + cpB 1 + pp 1 + tp 1 = 8 banks
        spool = ctx.enter_context(tc.tile_pool(name="spool", bufs=2, space="PSUM"))
        cpoolA = ctx.enter_context(tc.tile_pool(name="cpoolA", bufs=1, space="PSUM"))
        cpoolB = ctx.enter_context(tc.tile_pool(name="cpoolB", bufs=1, space="PSUM"))
        ppsum = ctx.enter_context(tc.tile_pool(name="ppsum", bufs=1, space="PSUM"))
        tpsum = ctx.enter_context(tc.tile_pool(name="tpsum", bufs=1, space="PSUM"))

        def load_x(xdram, tag, ch, bufs):
            """DMA a 512-token chunk of host-pretransposed bf16 X^T into SBUF.

            Single DMA instruction (3D access pattern) -- HWDGE is serialized
            at ~625ns per DMA instruction, so 1 beats 8 per-ft transfers.
            """
            tok0 = ch * CHUNK
            xt = xt_pool.tile(
                [P, FT * CHUNK], bf16, tag=f"xt{tag}", name=f"xt{tag}", bufs=bufs
            )
            xt = xt.rearrange("p (f l) -> p f l", f=FT)
            xdr = xdram.rearrange("(f p) l -> p f l", p=P)[:, :, tok0 : tok0 + CHUNK]
            # xparts DMA instructions per chunk: real-HW DMA rings parallelize
            # across instructions, so more (smaller) transfers can be faster
            # even though each costs ~625ns of HWDGE processing.
            np_ = cfg["xparts"] if ch > 0 else max(cfg["xparts"], 2)
            step = FT // np_
            for i in range(0, FT, step):
                nc.sync.dma_start(xt[:, i : i + step, :], xdr[:, i : i + step, :])
            return xt

        def proj_T_ct(wsb, xt, bsb, OUT, ch, ct, units=None):
            """^T-layout projection (columns on partitions) with bias."""
            tok0 = ch * CHUNK
            cell = {}

            def mms(fts):
                if "pp" not in cell:
                    cell["pp"] = ppsum.tile([P, CHUNK], f32, tag="pp", name="pp")
                for ft in fts:
                    nc.tensor.matmul(
                        cell["pp"][:],
                        wsb[:, ft, ct * P : (ct + 1) * P],
                        xt[:, ft, :],
                        start=(ft == 0),
                        stop=(ft == FT - 1),
                    )

            def fin():
                nc.vector.tensor_scalar_add(
                    OUT[:, ct, tok0 : tok0 + CHUNK], cell["pp"], bsb[:, ct : ct + 1]
                )

            if units is None:
                mms(range(FT))
                fin()
            else:
                for f0 in range(0, FT, 2):
                    units.append(lambda f=f0: mms((f, f + 1)))
                units.append(fin)

        def proj_v_it(xt, ch, it, units=None):
            """natural-layout V projection (tokens on partitions) with bias."""
            cell = {}

            def mms(fts):
                if "pv" not in cell:
                    cell["pv"] = ppsum.tile([P, COLS], f32, tag="pp", name="pv")
                for ft in fts:
                    nc.tensor.matmul(
                        cell["pv"][:],
                        xt[:, ft, it * P : (it + 1) * P],
                        wv_sb[:, ft, :],
                        start=(ft == 0),
                        stop=(ft == FT - 1),
                    )

            def fin():
                nc.vector.tensor_tensor(
                    V[:, ch * IT + it, :, 0:DK],
                    cell["pv"][:].rearrange("p (h w) -> p h w", h=HPC),
                    bv_bcast[:].rearrange("p (h w) -> p h w", h=HPC),
                    op=add_op,
                )

            if units is None:
                mms(range(FT))
                fin()
            else:
                units.append(lambda: mms(range(0, 4)))
                units.append(lambda: mms(range(4, FT)))
                units.append(fin)

        def qout_dma(ch):
            tok0 = ch * CHUNK
            if cfg["qoparts"] == 1:
                # one DMA for both column-tiles of the chunk
                nc.sync.dma_start(
                    q_outt.rearrange("(c p) l -> p c l", p=P)[
                        :, :, tok0 : tok0 + CHUNK
                    ],
                    QT[:, :, tok0 : tok0 + CHUNK].bitcast(f32),
                )
            else:
                for ct in range(CT):
                    nc.sync.dma_start(
                        q_outt[ct * P : (ct + 1) * P, tok0 : tok0 + CHUNK],
                        QT[:, ct, tok0 : tok0 + CHUNK].bitcast(f32),
                    )

        def kv_chunk(ch, units=None):
            xt = load_x(x_kvt, "kv", ch, cfg["kvbufs"])
            for ct in range(CT):
                proj_T_ct(wk_sb, xt, bk_sb, KT, ch, ct, units)
            for it in range(IT):
                proj_v_it(xt, ch, it, units)

        def q_chunk(ch, units=None):
            xt = load_x(x_qt, "q", ch, 2)
            for ct in range(CT):
                proj_T_ct(wq_sb, xt, bq_sb, QT, ch, ct, units)
            if units is None:
                qout_dma(ch)
            else:
                units.append(lambda: qout_dma(ch))

        def normalize(cp, h, ctx_tiles):
            """cp[:VW] = [ctx^T | sums] for head h -> scaled natural ctx."""
            ctT = ctpool.tile([P, CHUNK], f32, tag="ctT", name="ctT")
            nc.vector.tensor_copy(ctT[:VW, :], cp[:VW, :])
            for it in range(IT):
                t2 = tpsum.tile([P, CHUNK], f32, tag="tp", name="t2")
                nc.tensor.transpose(
                    t2[:, :VW],
                    ctT[:VW, it * P : (it + 1) * P],
                    identity[:VW, :VW],
                )
                rec = recpool.tile([P, 2], f32, tag="rec", name="rec")
                # (the reference's +1e-8 is dropped: sums are O(1000), so the
                # relative effect is <1e-11)
                nc.vector.reciprocal(rec[:, 1:2], t2[:, DK : DK + 1])
                nc.vector.tensor_scalar_mul(
                    ctx_tiles[it][:, h * DK : (h + 1) * DK],
                    t2[:, 0:DK],
                    rec[:, 1:2],
                )

        # MM2 backlog carried ACROSS ct/qc boundaries: the exp pipeline never
        # drains, so ACT stays fed and PE never waits out a refill.  Each
        # entry closes over its own (cp tiles, heads, q0); when a pass's last
        # entry (kt==TT-1) pops, its normalize chain is emitted.
        pend = []

        def pump(drain_until):
            ent = pend.pop(0)
            drain_until(ent["vf"])
            nc.tensor.matmul(
                ent["cpA"][:VW, :],
                V[:, ent["kt"], ent["hA"], :],
                ent["es"][:, 0:CHUNK],
                start=(ent["kt"] == 0),
                stop=(ent["kt"] == TT - 1),
            )
            nc.tensor.matmul(
                ent["cpB"][:VW, :],
                V[:, ent["kt"], ent["hB"], :],
                ent["es"][:, CHUNK : 2 * CHUNK],
                start=(ent["kt"] == 0),
                stop=(ent["kt"] == TT - 1),
            )
            if ent["kt"] == TT - 1:
                normalize(ent["cpA"], ent["hA"], ent["ctx_tiles"])
                normalize(ent["cpB"], ent["hB"], ent["ctx_tiles"])
                if ent["hB"] == 2 * CT - 1:
                    # last ct of this q-chunk: flush the natural-layout tiles
                    q0 = ent["q0"]
                    for it in range(IT):
                        nc.sync.dma_start(
                            c_out[q0 + it * P : q0 + (it + 1) * P, :],
                            ent["ctx_tiles"][it][:],
                        )

        def expify(es, sp, gi, qc):
            # Late q-chunks have no proj feed work on PE, so the attention
            # steady state is exp-bound: widen the DVE share from 1/4 to 3/8
            # there (measured relerr is unchanged -- the max-err element is
            # ACT-side).  dve_off keeps early DVE groups away from the
            # normalize bursts (also DVE) at kt=15.
            if qc >= cfg["late_from"]:
                on_dve = (gi % 8) in cfg["dve_late"]
            else:
                on_dve = NE and gi % NE == cfg["dve_off"]
            if on_dve:
                # DVE Schraudolph exp: bf16 bits via fused mul-add
                nc.vector.tensor_scalar(
                    es[:].bitcast(i16),
                    sp[:],
                    SCH_A,
                    SCH_B,
                    op0=mult_op,
                    op1=add_op,
                )
            else:
                nc.scalar.activation(es[:], sp[:], Exp, scale=0.125)

        def attention(qc, feed, drain_until, k_fin, v_fin, q_fin):
            q0 = qc * CHUNK
            ctx_tiles = [
                ctxpool.tile([P, COLS], f32, tag=f"ctx{it}", name=f"ctxt{it}")
                for it in range(IT)
            ]
            for ct in range(CT):
                hA, hB = 2 * ct, 2 * ct + 1
                drain_until(q_fin[qc][ct])
                cpA = cpoolA.tile([P, CHUNK], f32, tag="cpA", name="cpA")
                cpB = cpoolB.tile([P, CHUNK], f32, tag="cpB", name="cpB")
                for kt in range(TT):
                    # K^T for key-chunk kt//4, tile ct, must be emitted
                    drain_until(k_fin[kt // 4][ct])
                    sp = spool.tile([P, 2 * CHUNK], f32, tag="sp", name="sp")
                    # two concurrent row-tiled score matmuls (tile_position
                    # auto-derives (0,0) and (64,0) from base partitions)
                    nc.tensor.matmul(
                        sp[:, 0:CHUNK],
                        KT[0:DK, ct, kt * P : (kt + 1) * P],
                        QT[0:DK, ct, q0 : q0 + CHUNK],
                        start=True,
                        stop=True,
                    )
                    nc.tensor.matmul(
                        sp[:, CHUNK : 2 * CHUNK],
                        KT[DK:P, ct, kt * P : (kt + 1) * P],
                        QT[DK:P, ct, q0 : q0 + CHUNK],
                        start=True,
                        stop=True,
                    )
                    es = espool.tile([P, 2 * CHUNK], bf16, tag="es", name="es")
                    expify(es, sp, kt, qc)
                    pend.append(
                        dict(
                            kt=kt, es=es, cpA=cpA, cpB=cpB,
                            hA=hA, hB=hB, q0=q0, vf=v_fin[kt],
                            ctx_tiles=ctx_tiles,
                        )
                    )
                    if len(pend) > LAG:
                        pump(drain_until)
                    feed()

        def emit_all():
            units = []
            state = {"popped": 0}

            def pop1():
                units.pop(0)()
                state["popped"] += 1

            def feed():
                for _ in range(cfg["feedn"]):
                    if units:
                        pop1()

            def drain_until(idx):
                """Ensure the unit with absolute index idx has been emitted."""
                while idx is not None and state["popped"] <= idx:
                    pop1()

            def mark():
                return len(units) + state["popped"] - 1  # index of last appended

            # fill: K0 + Q0 eagerly (they gate the first scores/exp), V0 after.
            # DMA emission order = HWDGE order = transfer start order, so the
            # first matmul's operands (wk, x_kv chunk 0) go first.
            load_w(wk_sb, w_k)
            xt0 = load_x(x_kvt, "kv", 0, cfg["kvbufs"])
            load_w(wq_sb, w_q)
            xtq0 = load_x(x_qt, "q", 0, 2)
            load_w(wv_sb, w_v)
            load_biases()
            for ct in range(CT):
                proj_T_ct(wk_sb, xt0, bk_sb, KT, 0, ct)
            for ct in range(CT):
                proj_T_ct(wq_sb, xtq0, bq_sb, QT, 0, ct)
            qout_dma(0)
            for it in range(IT):
                proj_v_it(xt0, 0, it)
            # producer bookkeeping: fin indices (None = already emitted eagerly)
            k_fin = [[None, None] for _ in range(TCH)]
            v_fin = [None] * TT
            q_fin = [[None, None] for _ in range(TCH)]
            # feed queue: remaining KV chunks (all needed within attention
            # chunk 0), then Q chunks one-ahead.
            for ch in range(1, TCH):
                xt = load_x(x_kvt, "kv", ch, cfg["kvbufs"])
                for ct in range(CT):
                    proj_T_ct(wk_sb, xt, bk_sb, KT, ch, ct, units)
                    k_fin[ch][ct] = mark()
                for it in range(IT):
                    proj_v_it(xt, ch, it, units)
                    v_fin[ch * IT + it] = mark()
            for qc in range(TCH):
                nq = qc + 1
                if nq < TCH:
                    xt = load_x(x_qt, "q", nq, 2)
                    for ct in range(CT):
                        proj_T_ct(wq_sb, xt, bq_sb, QT, nq, ct, units)
                        q_fin[nq][ct] = mark()
                    units.append(lambda n=nq: qout_dma(n))
                attention(qc, feed, drain_until, k_fin, v_fin, q_fin)
            while pend:
                pump(drain_until)
            while units:
                pop1()

        for _rep in range(repeat):
            emit_all()
    nc.compile()
    return nc


def _get_nc():
    if "nc" not in _CACHE:
        _CACHE["nc"] = _build()
    return _CACHE["nc"]


def make_in_maps(cdd, his, W_q, b_q, W_k, b_k, W_v, b_v):
    cdd = np.asarray(cdd, dtype=np.float32)
    his = np.asarray(his, dtype=np.float32)
    W_q = np.asarray(W_q, dtype=np.float32)
    W_k = np.asarray(W_k, dtype=np.float32)
    W_v = np.asarray(W_v, dtype=np.float32)
    b_q = np.asarray(b_q, dtype=np.float32)
    b_k = np.asarray(b_k, dtype=np.float32)
    b_v = np.asarray(b_v, dtype=np.float32)
    import ml_dtypes

    bf16 = ml_dtypes.bfloat16
    # feature-major bf16 inputs, shared across the 4 cores of each batch
    cddT = [np.ascontiguousarray(cdd[b].T).astype(bf16) for b in range(B)]
    hisT = [np.ascontiguousarray(his[b].T).astype(bf16) for b in range(B)]
    W_q = W_q.astype(bf16)
    W_k = W_k.astype(bf16)
    W_v = W_v.astype(bf16)
    in_maps = []
    for c in range(NCORES):
        b, hb = divmod(c, CPB)
        sl = slice(hb * COLS, (hb + 1) * COLS)
        in_maps.append(
            {
                "x_qt": cddT[b],
                "x_kvt": hisT[b],
                "w_q": np.ascontiguousarray(W_q[:, sl]),
                "w_k": np.ascontiguousarray(W_k[:, sl]),
                "w_v": np.ascontiguousarray(W_v[:, sl]),
                "b_q": np.ascontiguousarray(b_q[sl]),
                "b_k": np.ascontiguousarray(b_k[sl]),
                "b_v": np.ascontiguousarray(b_v[sl]),
            }
        )
    return in_maps


def assemble_outputs(results):
    context = np.zeros((B, L, D), dtype=np.float32)
    q_a = np.zeros((B, L, D), dtype=np.float32)
    for c, out in enumerate(results):
        b, hb = divmod(c, CPB)
        sl = slice(hb * COLS, (hb + 1) * COLS)
        q_a[b, :, sl] = out["q_outt"].T
        context[b, :, sl] = out["c_out"]
    return (context, q_a)


def kernel(cdd, his, W_q, b_q, W_k, b_k, W_v, b_v):
    from concourse.bass_utils import run_bass_kernel_spmd

    nc = _get_nc()
    in_maps = make_in_maps(cdd, his, W_q, b_q, W_k, b_k, W_v, b_v)

    res = run_bass_kernel_spmd(
        nc, in_maps, core_ids=list(range(NCORES)), trace=_CACHE.get("trace", False)
    )
    _CACHE["last_result"] = res
    return assemble_outputs(res.results)

